# revision 38
# baseline (speedup 1.0000x reference)
import sys

import numpy as np

sys.path.insert(0, "/opt/trn_rl_repo")

import concourse.tile as tile
from concourse import bacc, bass, mybir
from concourse.bass_utils import run_bass_kernel_spmd
from concourse.masks import make_identity

P = 128
BATCH = 4096
DIM = 300
VOCAB = 50000
NCORES = 8
VSH = VOCAB // NCORES  # 6250 vocab columns per core
NB = BATCH // P  # 32 batch tiles
KSZ = [128, 128, 44]
KOF = [0, 128, 256]
# 13 column pieces per batch tile; each streams out as its own DMA on a
# rotating queue (SP/ACT/Pool overlap at full bandwidth on the DMA pool).
# all piece widths >= 256 so float32r matmul runs at 1 cycle/row.
# widths kept even: birsim rejects odd-width f32r matmuls (ISA check)
PIECE_W = [512] * 11 + [310, 308]
PIECE_O = [0]
for _w in PIECE_W[:-1]:
    PIECE_O.append(PIECE_O[-1] + _w)
NP_ = len(PIECE_W)
# last batch tile: split the final 308 piece into 256+52 so the drain
# tail after the last matmul is one tiny DMA instead of a 308-wide one
PIECE_W_LAST = PIECE_W[:-1] + [256, 52]
PIECE_O_LAST = [0]
for _w in PIECE_W_LAST[:-1]:
    PIECE_O_LAST.append(PIECE_O_LAST[-1] + _w)
N_WARM = 28
F32 = mybir.dt.float32
F32R = mybir.dt.float32r
I32 = mybir.dt.int32


def _build():
    # Bacc's compile() runs generate_event_semaphores, splitting multi-sem
    # waits into EventSemaphore instructions -- real HW allows only one
    # embedded sync wait on Matmult and DMA descriptors
    nc = bacc.Bacc(None, target_bir_lowering=False)
    x = nc.declare_dram_parameter("x", [P, NB], I32, isOutput=False)
    w_encT = nc.declare_dram_parameter("w_encT", [VOCAB, DIM], F32R, isOutput=False)
    w_decT = nc.declare_dram_parameter("w_decT", [DIM, VSH], F32R, isOutput=False)
    out = nc.declare_dram_parameter("out", [BATCH, VSH], F32, isOutput=True)

    with tile.TileContext(nc) as tc:
        with (
            tc.tile_pool(name="persist", bufs=1) as persist,
            tc.tile_pool(name="work", bufs=4) as work,
            tc.tile_pool(name="stage", bufs=20) as stage_pool,
            tc.tile_pool(name="psum", bufs=1, space="PSUM") as psum_pool,
        ):
            id_f32 = work.tile([P, P], F32, tag="idf32")
            make_identity(nc, id_f32[:])
            # f32r identity makes transposes 1.5 cyc/row instead of 2.0;
            # produce it via DVE copy to satisfy the fp32r verifier
            identity = persist.tile([P, P], F32R, tag="identity")
            nc.vector.tensor_copy(out=identity[:], in_=id_f32[:])
            # dummy transposes keep PE busy while the first gather is in
            # flight, so the pstate ramp completes before real work starts.
            # all dummies write one non-rotating psum tile: no readers means
            # each dummy carries exactly one sem wait (HW allows only one
            # sync wait per Matmult)
            warm = psum_pool.tile([P, P], F32R, tag="warm", bufs=1, name="warm")
            for _ in range(N_WARM):
                nc.tensor.transpose(
                    out=warm[:], in_=identity[:], identity=identity[:]
                )
            idx_all = persist.tile([P, NB], I32, tag="idx")
            nc.gpsimd.dma_start(out=idx_all[:], in_=x[:, :])
            hT = [
                persist.tile([P, BATCH], F32R, tag=f"hT{k}", name=f"hT{k}")
                for k in range(3)
            ]
            wd = [
                persist.tile([P, VSH], F32R, tag=f"wd{k}", name=f"wd{k}")
                for k in range(3)
            ]

            def load_wd(eng, k, p):
                c0, cw = PIECE_O[p], PIECE_W[p]
                eng.dma_start(
                    out=wd[k][: KSZ[k], c0 : c0 + cw],
                    in_=w_decT[KOF[k] : KOF[k] + KSZ[k], c0 : c0 + cw],
                )

            # k0 on SP queue, k1 on ACT queue, k2 on Pool queue (split across
            # bt0/bt1 so gathers interleave) -- three queues load in parallel
            for p in range(NP_):
                load_wd(nc.sync, 0, p)
            for p in range(NP_):
                load_wd(nc.scalar, 1, p)

            out_eng = [nc.sync, nc.scalar, nc.gpsimd]
            h_tiles = {}

            def emit_gather(bt):
                h = work.tile([P, DIM], F32R, tag="h", name="h")
                nc.gpsimd.indirect_dma_start(
                    out=h[:],
                    out_offset=None,
                    in_=w_encT[:, :],
                    in_offset=bass.IndirectOffsetOnAxis(
                        ap=idx_all[:, bt : bt + 1], axis=0
                    ),
                )
                h_tiles[bt] = h

            emit_gather(0)
            emit_gather(1)
            emit_gather(2)
            for p in range(NP_):
                load_wd(nc.gpsimd, 2, p)

            def emit_transposes(bt):
                h = h_tiles.pop(bt)
                for k in range(3):
                    pt = psum_pool.tile([P, P], F32R, tag="tp", bufs=2, name="pt")
                    nc.tensor.transpose(
                        out=pt[: KSZ[k], :],
                        in_=h[:, KOF[k] : KOF[k] + KSZ[k]],
                        identity=identity[:],
                    )
                    # ACT/DVE copy rounds fp32 -> fp32r (verifier wants
                    # matmul operands produced as fp32r)
                    dst = hT[k][: KSZ[k], bt * P : (bt + 1) * P]
                    if bt >= 2 and k == 1:
                        nc.scalar.copy(out=dst, in_=pt[: KSZ[k], :])
                    else:
                        nc.vector.tensor_copy(out=dst, in_=pt[: KSZ[k], :])

            emit_transposes(0)
            for bt in range(NB):
                pw = PIECE_W if bt + 1 < NB else PIECE_W_LAST
                po = PIECE_O if bt + 1 < NB else PIECE_O_LAST
                for g in range(len(pw)):
                    c0, nv = po[g], pw[g]
                    ps = psum_pool.tile([P, 512], F32, tag="mm", bufs=5, name="ps")
                    for k in range(3):
                        nc.tensor.matmul(
                            ps[:, :nv],
                            hT[k][: KSZ[k], bt * P : (bt + 1) * P],
                            wd[k][: KSZ[k], c0 : c0 + nv],
                            start=(k == 0),
                            stop=(k == 2),
                        )
                    st = stage_pool.tile([P, 512], F32, tag="st", name="st")
                    # ACT groups exclude g8: with 5 psum bufs, g8's st copy
                    # is the psum-buffer reader for the next bt's g0, whose
                    # k0 matmul already waits on DVE (hT0) and can't carry a
                    # second sync wait (HW limit: one per Matmult)
                    if bt >= 2 and g in (1, 4, 7, 10):
                        nc.scalar.copy(out=st[:, :nv], in_=ps[:, :nv])
                    else:
                        nc.vector.tensor_copy(out=st[:, :nv], in_=ps[:, :nv])
                    out_eng[g % 3].dma_start(
                        out=out[bt * P : (bt + 1) * P, c0 : c0 + nv],
                        in_=st[:, :nv],
                    )
                    if g == 6:
                        if bt + 1 < NB:
                            emit_transposes(bt + 1)
                        if bt + 3 < NB:
                            emit_gather(bt + 3)
    nc.compile()
    return nc


_NC_CACHE = None


def _get_nc():
    global _NC_CACHE
    if _NC_CACHE is None:
        _NC_CACHE = _build()
    return _NC_CACHE


def _prep_in_maps(x, w_enc, w_dec):
    xt = np.asarray(x).astype(np.int32).reshape(NB, P).T
    x32 = np.ascontiguousarray(xt)
    w_encT = np.ascontiguousarray(np.asarray(w_enc, dtype=np.float32).T)
    w_dec = np.asarray(w_dec, dtype=np.float32)
    in_maps = []
    for m in range(NCORES):
        wdT = np.ascontiguousarray(w_dec[m * VSH : (m + 1) * VSH, :].T)
        in_maps.append({"x": x32, "w_encT": w_encT, "w_decT": wdT})
    return in_maps


def run(x, w_enc, w_dec, trace=False):
    nc = _get_nc()
    res = run_bass_kernel_spmd(
        nc,
        _prep_in_maps(x, w_enc, w_dec),
        core_ids=list(range(NCORES)),
        trace=trace,
    )
    outs = [np.asarray(r["out"]) for r in res.results]
    return np.concatenate(outs, axis=1), res


def kernel(**inputs):
    out, _ = run(inputs["x"], inputs["w_enc"], inputs["w_dec"])
    return out
